# revision 17
# baseline (speedup 1.0000x reference)
"""GCN block (2-layer) Trainium2 Bass kernel, v2.

Math (per B*T slice, shared graph):
  t2 = relu(A @ (X @ W1) + b1);  out = sigmoid(A @ t2 @ W2 + b2)
  A = D^-1/2 (Adj + I) D^-1/2  (PyG gcn_norm, counts edge multiplicity)

Device mapping (all-fp8 PE pipeline, M = Adj + I exact small ints in fp8):
  W1 : stationary = X-blocks (fp8, dinv-src and 1/8-of-W1-scale folded),
       moving = blockdiag(8*W1) fp8; psum drains straight into node-major
       fp8 pair tiles in SBUF (no DRAM round trip).
  L1 : normal orientation - stationary = MT column-slabs (viewed as
       M-row blocks), moving = xw pair tiles, fp8 DoubleRow K=256;
       drain folds dinv_dst, bias, relu, and next-layer dinv_src.
  t2 : distributed incrementally - one AllGather per dst-block pair
       (5 checkpoints) so the exchange overlaps the L1 compute.
  L2 : FLIPPED orientation - stationary = t2 pair tiles (node-major),
       moving = MT dst-chunk slabs, fp8 DoubleRow; output lands
       feature-major [sf, dst] so W2 + sigmoid fuse directly with no
       transposes and no DRAM staging of s2.
  W2 : stationary blockdiag(W2) bf16 over the drained bf16 s2 chunks,
       sigmoid+bias on ACT, fp32 tiles DMA'd to the output.

Sharding: each of 8 cores owns 10 of the 80 dst-node blocks (N padded
10000->10240) for ALL 24 B*T slices.  SBUF keeps the full xw / t2
operand set resident (40 pair tiles, 120 KB/partition); the same ring
is reused between layers (t2c[j2] overwrites xwp[j2]).
"""
import time

import numpy as np
import ml_dtypes

import concourse.bacc as bacc
import concourse.mybir as mybir
import concourse.tile as tile
from concourse.bass_utils import run_bass_kernel_spmd

N_CORES = 8
N = 10000
NP = 10240            # padded nodes
NB = NP // 128        # 80 node blocks
NB2 = NB // 2         # 40 src-block pairs (DoubleRow K=256)
BPC = NB // N_CORES   # 10 dst blocks per core
NCK = BPC // 2        # 5 t2 checkpoints (dst-block pairs) per core
CHW = 256             # L2 dst-chunk width
NCH = BPC * 128 // CHW  # 5 dst chunks per core
B, T, C = 2, 12, 64
S = B * T             # 24 slices
F = S * C             # 1536 free columns
PAIRS = S // 2        # 12 slice pairs (pl)
CHAINS = ((0, 512), (512, 512), (1024, 512))
W1SCALE = 8.0         # W1 pre-scale so fp8 weights stay mostly normal
DSPL = 704            # W1 drain split: DVE [0:DSPL], ACT [DSPL:F] balanced

f32 = mybir.dt.float32
bf16 = mybir.dt.bfloat16
fp8 = mybir.dt.float8e4
DR = mybir.MatmulPerfMode.DoubleRow
AF = mybir.ActivationFunctionType


def build_program(with_collective=True, nc_hook=None):
    nc = bacc.Bacc("TRN2", target_bir_lowering=False, debug=False,
                   num_devices=N_CORES)
    if nc_hook is not None:
        nc_hook(nc)

    # X blocks: [b][128=(h,cin)][pl*128+node] fp8, dinv-src/W1SCALE folded
    xb_ext = nc.dram_tensor("XB8", [NB, 128, PAIRS * 128], fp8,
                            kind="ExternalInput")
    # MT column slabs: [chunk][128 src][j2*512 + k*256 + dst] fp8 ints
    mt_ext = nc.dram_tensor("MT", [NCH, 128, NB2 * 2 * CHW], fp8,
                            kind="ExternalInput")
    w1_ext = nc.dram_tensor("W1d", [128, 128], fp8, kind="ExternalInput")
    w2_ext = nc.dram_tensor("W2d", [128, 128], bf16, kind="ExternalInput")
    b1_ext = nc.dram_tensor("B1", [128, F], f32, kind="ExternalInput")
    b2_ext = nc.dram_tensor("B2", [128, 1], f32, kind="ExternalInput")
    di_ext = nc.dram_tensor("DI", [128, BPC], f32, kind="ExternalInput")
    d8_ext = nc.dram_tensor("DI8", [128, BPC], f32, kind="ExternalInput")
    db_ext = nc.dram_tensor("DB", [128, BPC * 128], f32,
                            kind="ExternalInput")
    out_ext = nc.dram_tensor("OUT", [PAIRS, 128, BPC * 128], f32,
                             kind="ExternalOutput")

    with tile.TileContext(nc) as tc:
        with (
            tc.tile_pool(name="consts", bufs=1) as consts,
            tc.tile_pool(name="xb", bufs=3) as pool_xb,
            tc.tile_pool(name="xwp", bufs=NB2) as pool_xwp,
            tc.tile_pool(name="mt", bufs=2) as pool_mt,
            tc.tile_pool(name="u", bufs=3) as pool_u,
            tc.tile_pool(name="t2s", bufs=2) as pool_t2s,
            tc.tile_pool(name="s2", bufs=3) as pool_s2,
            tc.tile_pool(name="outst", bufs=2) as pool_out,
            tc.tile_pool(name="ps", bufs=8, space="PSUM") as pool_ps,
            tc.tile_pool(name="dram", bufs=1, space="DRAM") as dram,
        ):
            # constants
            w1t = consts.tile([128, 128], fp8, tag="w1")
            nc.sync.dma_start(w1t[:], w1_ext[:])
            w2t = consts.tile([128, 128], bf16, tag="w2")
            nc.sync.dma_start(w2t[:], w2_ext[:])
            b1t = consts.tile([128, F], f32, tag="b1")
            nc.sync.dma_start(b1t[:], b1_ext[:])
            b2t = consts.tile([128, 1], f32, tag="b2")
            nc.sync.dma_start(b2t[:], b2_ext[:])
            dit = consts.tile([128, BPC], f32, tag="di")
            nc.sync.dma_start(dit[:], di_ext[:])
            di8 = consts.tile([128, BPC], f32, tag="di8")
            nc.sync.dma_start(di8[:], d8_ext[:])
            dbt = consts.tile([128, BPC * 128], f32, tag="db")
            nc.sync.dma_start(dbt[:], db_ext[:])

            # DRAM intermediates: per-checkpoint t2 slabs
            t2loc = [dram.tile([2, 128, F], fp8, tag="t2loc",
                               name=f"t2loc{k}") for k in range(NCK)]
            if with_collective:
                t2full = [dram.tile([N_CORES, 2, 128, F], fp8, tag="t2full",
                                    name=f"t2full{k}", addr_space="Shared")
                          for k in range(NCK)]
            else:
                t2full = [dram.tile([N_CORES, 2, 128, F], fp8, tag="t2full",
                                    name=f"t2full{k}") for k in range(NCK)]

            # Checkpoint-major step order: pairs of checkpoint k (j2 % NCK
            # == k) are produced / exchanged / consumed first, so the xwp
            # ring slots, the t2c reads, and the L2 chains all pipeline in
            # the same order.
            ORD = [k + NCK * i for k in range(NCK) for i in range(NB2 // NCK)]

            # ---- W1: xw pair tiles, node-major, fp8 (stays in SBUF) ----
            # All 8 PSUM banks rotate as [128,512] tiles so the
            # mm->drain->reuse cycle (~1.2us) is fully hidden; drains
            # interleave DVE/ACT at a 7:9 ratio to balance the engines.
            # xwp tiles are ALLOCATED in ORD order: ring releases fire in
            # allocation order, and L1 consumes pairs in ORD order, so the
            # t2c generation can reuse slots progressively.
            xwp = [None] * NB2
            for i in range(NB2):
                xwp[ORD[i]] = pool_xwp.tile([128, 2, F], fp8, tag="xwp",
                                            name=f"xwp{ORD[i]}")
            nchunk = 0
            for j2 in range(NB2):
                xb = pool_xb.tile([128, 2, F], fp8, tag="xb",
                                  name=f"xb{j2}")
                nc.sync.dma_start(
                    xb[:], xb_ext[2 * j2:2 * j2 + 2]
                    .rearrange("a p d -> p a d"))
                xw = xwp[j2]
                for k in range(2):
                    for c3 in range(3):
                        ps = pool_ps.tile([128, 512], f32, tag="ps",
                                          name=f"w1p{nchunk}")
                        for p4 in range(4):
                            pl = 4 * c3 + p4
                            nc.tensor.matmul(
                                ps[:, p4 * 128:(p4 + 1) * 128],
                                xb[:, k, pl * 128:(pl + 1) * 128], w1t[:],
                                start=True, stop=True)
                        dst = xw[:, k, c3 * 512:(c3 + 1) * 512]
                        if (nchunk * 7) % 16 < 7:
                            nc.vector.tensor_scalar_mul(dst, ps[:], 1.0)
                        else:
                            nc.scalar.activation(dst, ps[:], AF.Copy)
                        nchunk += 1

            # ---- MT chunk loads (ring bufs=2; ch0/1 prefetch for L1) ----
            mts = []
            for ch in range(NCH):
                mtt = pool_mt.tile([128, NB2, 2, CHW], fp8, tag="mt",
                                   name=f"mt{ch}")
                nc.sync.dma_start(
                    mtt[:].rearrange("p a b d -> p (a b d)"), mt_ext[ch])
                mts.append(mtt)

            # ---- L1: t2 = dinv*relu(dinv/8*(M @ xw) + b1), 5 checkpoints
            # Each checkpoint runs its 2 blocks x 3 chains as 6 live psum
            # chains, pair-inner and skewed, so xwp slots free across the
            # whole pass and chain endings stagger.
            for ch in range(NCK):
                t2st = pool_t2s.tile([128, 2, F], fp8, tag="t2s",
                                     name=f"t2s{ch}")
                ps_list = [pool_ps.tile([128, 512], f32, tag="ps",
                                        name=f"pa{ch}_{idx}")
                           for idx in range(6)]
                for r in range(NB2 + 6):
                    for idx in range(6):
                        i = r - idx
                        if not 0 <= i < NB2:
                            continue
                        j2 = ORD[i]
                        h2, (c0, w) = idx // 3, CHAINS[idx % 3]
                        nc.tensor.matmul(
                            ps_list[idx][:],
                            mts[ch][:, j2, :, h2 * 128:(h2 + 1) * 128],
                            xwp[j2][:, :, c0:c0 + w],
                            start=(i == 0), stop=(i == NB2 - 1),
                            perf_mode=DR)
                    idx = r - (NB2 - 1)
                    if 0 <= idx < 6:
                        h2, (c0, w) = idx // 3, CHAINS[idx % 3]
                        bi = 2 * ch + h2
                        u = pool_u.tile([128, 512], f32, tag="u")
                        nc.vector.scalar_tensor_tensor(
                            u[:], ps_list[idx][:], di8[:, bi:bi + 1],
                            b1t[:, c0:c0 + w],
                            mybir.AluOpType.mult, mybir.AluOpType.add)
                        nc.scalar.activation(t2st[:, h2, c0:c0 + w], u[:],
                                             AF.Relu,
                                             scale=dit[:, bi:bi + 1])
                # distribute this checkpoint's 2 blocks
                nc.gpsimd.dma_start(
                    t2loc[ch][:].rearrange("a p f -> p a f"), t2st[:])
                if with_collective:
                    nc.gpsimd.collective_compute(
                        "AllGather", mybir.AluOpType.bypass,
                        replica_groups=[list(range(N_CORES))],
                        ins=[t2loc[ch][:]], outs=[t2full[ch][:]])
                else:
                    for c in range(N_CORES):
                        nc.gpsimd.dma_start(t2full[ch][c], t2loc[ch][:])

            # ---- MT reload for L2 ch3 first (its slot frees mid-L1) ----
            mtb = {NCH - 1: mts[NCH - 1]}
            t = pool_mt.tile([128, NB2, 2, CHW], fp8, tag="mt",
                             name=f"mtb{NCH - 2}")
            nc.sync.dma_start(
                t[:].rearrange("p a b d -> p (a b d)"), mt_ext[NCH - 2])
            mtb[NCH - 2] = t

            # ---- t2c pair tiles (reuse the xwp ring: t2c[j2] -> slot j2;
            # DMAs issued in ORD order so reads chase the slot releases
            # and the checkpoint arrivals in lock-step) ----
            t2c = [pool_xwp.tile([128, 2, F], fp8, tag="xwp",
                                 name=f"t2c{j2}") for j2 in range(NB2)]
            for i in range(NB2):
                j2 = ORD[i]
                gc = 2 * j2
                core, ck = gc // BPC, (gc % BPC) // 2
                nc.sync.dma_start(
                    t2c[j2][:], t2full[ck][core].rearrange("a p f -> p a f"))

            # ---- remaining MT reloads (descending; ch4 still resident) ----
            for ch in range(NCH - 3, -1, -1):
                t = pool_mt.tile([128, NB2, 2, CHW], fp8, tag="mt",
                                 name=f"mtb{ch}")
                nc.sync.dma_start(
                    t[:].rearrange("p a b d -> p (a b d)"), mt_ext[ch])
                mtb[ch] = t

            # ---- L2 (flipped) + W2 + sigmoid, per dst chunk ----
            # Chains run skewed (chain pl lags pl rounds) so chain endings
            # stagger and the drain/W2/sigmoid tail pipelines instead of
            # serializing after the whole chunk.
            for ch in range(NCH - 1, -1, -1):
                mtt = mtb[ch]
                wt = [pool_ps.tile([128, 512], f32, tag="ps",
                                   name=f"l2w{ch}_{i}") for i in range(6)]
                outst = None
                for r in range(NB2 + PAIRS):
                    for pl in range(PAIRS):
                        i = r - pl
                        if not 0 <= i < NB2:
                            continue
                        j2 = ORD[i]
                        nc.tensor.matmul(
                            wt[pl // 2][:, (pl % 2) * CHW:(pl % 2 + 1) * CHW],
                            t2c[j2][:, :, pl * 128:(pl + 1) * 128],
                            mtt[:, j2, :, :],
                            start=(i == 0), stop=(i == NB2 - 1),
                            perf_mode=DR)
                    pl = r - (NB2 - 1)
                    if 0 <= pl < PAIRS:
                        s2 = pool_s2.tile([128, CHW], bf16, tag="s2")
                        nc.vector.tensor_tensor(
                            s2[:],
                            wt[pl // 2][:, (pl % 2) * CHW:(pl % 2 + 1) * CHW],
                            dbt[:, ch * CHW:(ch + 1) * CHW],
                            mybir.AluOpType.mult)
                        ps2 = pool_ps.tile([128, 512], f32, tag="ps",
                                           name=f"w2p{ch}_{pl}")
                        nc.tensor.matmul(ps2[:, :CHW], w2t[:], s2[:],
                                         start=True, stop=True)
                        if pl % 3 == 0:
                            outst = pool_out.tile([128, 3, CHW], f32,
                                                  tag="outst")
                        nc.scalar.activation(outst[:, pl % 3, :],
                                             ps2[:, :CHW],
                                             AF.Sigmoid, bias=b2t[:])
                        if pl % 3 == 2:
                            nc.gpsimd.dma_start(
                                out_ext[pl - 2:pl + 1, :,
                                        ch * CHW:(ch + 1) * CHW]
                                .rearrange("a p d -> p a d"), outst[:])

    nc.compile()
    return nc


def prepare_inputs(X, edge_index, W1, b1, W2, b2):
    """Host-side graph/layout prep. Returns per-core in_maps."""
    X = np.asarray(X, dtype=np.float32)
    edge_index = np.asarray(edge_index)
    W1 = np.asarray(W1, dtype=np.float32)
    b1 = np.asarray(b1, dtype=np.float32)
    W2 = np.asarray(W2, dtype=np.float32)
    b2 = np.asarray(b2, dtype=np.float32)

    src = edge_index[0].astype(np.int64)
    dst = edge_index[1].astype(np.int64)

    deg = np.bincount(dst, minlength=N).astype(np.float32) + 1.0
    dinv = 1.0 / np.sqrt(deg)
    dinv_pad = np.zeros(NP, np.float32)
    dinv_pad[:N] = dinv

    # M = Adj + I with multiplicity, uint8 counts
    Mfull = np.zeros((NP, NP), np.uint8)
    np.add.at(Mfull, (dst, src), 1)
    Mfull[np.arange(N), np.arange(N)] += 1
    assert Mfull.max() <= 15, "fp8e4 exact-int range exceeded"

    # XB: [NB, 128=(h,cin), PAIRS*128] fp8 with dinv-src folded; s = 2*pl+h
    Xs = X * dinv[None, :, None, None]                  # [B, N, T, C]
    XT = np.zeros((S, C, NP), np.float32)
    XT[:, :, :N] = np.transpose(Xs, (0, 2, 3, 1)).reshape(S, C, N)
    x6 = XT.reshape(PAIRS, 2, C, NB, 128)
    XB = np.ascontiguousarray(np.transpose(x6, (3, 1, 2, 0, 4)))
    XB = XB.reshape(NB, 128, PAIRS * 128).astype(ml_dtypes.float8_e4m3)

    def blockdiag(W, dtype):
        D = np.zeros((128, 128), np.float32)
        D[:64, :64] = W
        D[64:, 64:] = W
        return D.astype(dtype)

    W1d = blockdiag(W1 * W1SCALE, ml_dtypes.float8_e4m3)
    W2d = blockdiag(W2, ml_dtypes.bfloat16)
    B1 = np.tile(b1, (128, F // C)).astype(np.float32)
    B2 = np.concatenate([b2, b2])[:, None].astype(np.float32)

    in_maps = []
    for c in range(N_CORES):
        # MT slab: [NCH][128 src][j2][k][dst-chunk] fp8 ints
        MTc = Mfull[c * BPC * 128:(c + 1) * BPC * 128, :].T  # [NP src, 1280]
        MTc = MTc.reshape(NB2, 2, 128, NCH, CHW)
        MTc = np.ascontiguousarray(np.transpose(MTc, (3, 2, 0, 1, 4)))
        MTc = MTc.reshape(NCH, 128, NB2 * 2 * CHW)
        MTc = MTc.astype(ml_dtypes.float8_e4m3)

        DIc = dinv_pad[c * BPC * 128:(c + 1) * BPC * 128]
        DI = np.ascontiguousarray(DIc.reshape(BPC, 128).T.astype(np.float32))
        DI8 = np.ascontiguousarray(DI / W1SCALE)
        DB = np.ascontiguousarray(
            np.tile(DIc[None, :], (128, 1)).astype(np.float32))
        in_maps.append({"XB8": XB, "MT": MTc, "W1d": W1d, "W2d": W2d,
                        "B1": B1, "B2": B2, "DI": DI, "DI8": DI8,
                        "DB": DB})
    return in_maps


_NC_CACHE = {}


def kernel(X, edge_index, W1, b1, W2, b2):
    if "nc" not in _NC_CACHE:
        _NC_CACHE["nc"] = build_program(with_collective=True)
    nc = _NC_CACHE["nc"]
    in_maps = prepare_inputs(X, edge_index, W1, b1, W2, b2)

    res = None
    for attempt in range(5):
        try:
            res = run_bass_kernel_spmd(nc, in_maps, list(range(N_CORES)))
            break
        except Exception:
            if attempt == 4:
                raise
            time.sleep(60.0 * (attempt + 1))
    assert res is not None

    # reassemble: per core [12, 128, 1280] -> [24, 64, 1280]
    full = np.zeros((S, C, N), np.float32)
    for c in range(N_CORES):
        o = res.results[c]["OUT"].reshape(S, C, BPC * 128)
        lo = c * BPC * 128
        hi = min(N, (c + 1) * BPC * 128)
        if lo < N:
            full[:, :, lo:hi] = o[:, :, :hi - lo]
    out = full.reshape(B, T, C, N).transpose(0, 3, 1, 2)
    return np.ascontiguousarray(out)


# revision 18
# speedup vs baseline: 1.0102x; 1.0102x over previous
"""GCN block (2-layer) Trainium2 Bass kernel, v2.

Math (per B*T slice, shared graph):
  t2 = relu(A @ (X @ W1) + b1);  out = sigmoid(A @ t2 @ W2 + b2)
  A = D^-1/2 (Adj + I) D^-1/2  (PyG gcn_norm, counts edge multiplicity)

Device mapping (all-fp8 PE pipeline, M = Adj + I exact small ints in fp8):
  W1 : stationary = X-blocks (fp8, dinv-src and 1/8-of-W1-scale folded),
       moving = blockdiag(8*W1) fp8; psum drains straight into node-major
       fp8 pair tiles in SBUF (no DRAM round trip).
  L1 : normal orientation - stationary = MT column-slabs (viewed as
       M-row blocks), moving = xw pair tiles, fp8 DoubleRow K=256;
       drain folds dinv_dst, bias, relu, and next-layer dinv_src.
  t2 : distributed incrementally - one AllGather per dst-block pair
       (5 checkpoints) so the exchange overlaps the L1 compute.
  L2 : FLIPPED orientation - stationary = t2 pair tiles (node-major),
       moving = MT dst-chunk slabs, fp8 DoubleRow; output lands
       feature-major [sf, dst] so W2 + sigmoid fuse directly with no
       transposes and no DRAM staging of s2.
  W2 : stationary blockdiag(W2) bf16 over the drained bf16 s2 chunks,
       sigmoid+bias on ACT, fp32 tiles DMA'd to the output.

Sharding: each of 8 cores owns 10 of the 80 dst-node blocks (N padded
10000->10240) for ALL 24 B*T slices.  SBUF keeps the full xw / t2
operand set resident (40 pair tiles, 120 KB/partition); the same ring
is reused between layers (t2c[j2] overwrites xwp[j2]).
"""
import time

import numpy as np
import ml_dtypes

import concourse.bacc as bacc
import concourse.mybir as mybir
import concourse.tile as tile
from concourse.bass_utils import run_bass_kernel_spmd

N_CORES = 8
N = 10000
NP = 10240            # padded nodes
NB = NP // 128        # 80 node blocks
NB2 = NB // 2         # 40 src-block pairs (DoubleRow K=256)
BPC = NB // N_CORES   # 10 dst blocks per core
NCK = BPC // 2        # 5 t2 checkpoints (dst-block pairs) per core
CHW = 256             # L2 dst-chunk width
NCH = BPC * 128 // CHW  # 5 dst chunks per core
B, T, C = 2, 12, 64
S = B * T             # 24 slices
F = S * C             # 1536 free columns
PAIRS = S // 2        # 12 slice pairs (pl)
CHAINS = ((0, 512), (512, 512), (1024, 512))
W1SCALE = 8.0         # W1 pre-scale so fp8 weights stay mostly normal
DSPL = 704            # W1 drain split: DVE [0:DSPL], ACT [DSPL:F] balanced

f32 = mybir.dt.float32
bf16 = mybir.dt.bfloat16
fp8 = mybir.dt.float8e4
DR = mybir.MatmulPerfMode.DoubleRow
AF = mybir.ActivationFunctionType


def build_program(with_collective=True, nc_hook=None):
    nc = bacc.Bacc("TRN2", target_bir_lowering=False, debug=False,
                   num_devices=N_CORES)
    if nc_hook is not None:
        nc_hook(nc)

    # X blocks: [b][128=(h,cin)][pl*128+node] fp8, dinv-src/W1SCALE folded
    xb_ext = nc.dram_tensor("XB8", [NB, 128, PAIRS * 128], fp8,
                            kind="ExternalInput")
    # MT column slabs: [chunk][128 src][j2*512 + k*256 + dst] fp8 ints
    mt_ext = nc.dram_tensor("MT", [NCH, 128, NB2 * 2 * CHW], fp8,
                            kind="ExternalInput")
    w1_ext = nc.dram_tensor("W1d", [128, 128], fp8, kind="ExternalInput")
    w2_ext = nc.dram_tensor("W2d", [128, 128], bf16, kind="ExternalInput")
    b1_ext = nc.dram_tensor("B1", [128, F], f32, kind="ExternalInput")
    b2_ext = nc.dram_tensor("B2", [128, 1], f32, kind="ExternalInput")
    di_ext = nc.dram_tensor("DI", [128, BPC], f32, kind="ExternalInput")
    d8_ext = nc.dram_tensor("DI8", [128, BPC], f32, kind="ExternalInput")
    db_ext = nc.dram_tensor("DB", [128, BPC * 128], f32,
                            kind="ExternalInput")
    out_ext = nc.dram_tensor("OUT", [PAIRS, 128, BPC * 128], f32,
                             kind="ExternalOutput")

    with tile.TileContext(nc) as tc:
        with (
            tc.tile_pool(name="consts", bufs=1) as consts,
            tc.tile_pool(name="xb", bufs=3) as pool_xb,
            tc.tile_pool(name="xwp", bufs=NB2) as pool_xwp,
            tc.tile_pool(name="mt", bufs=2) as pool_mt,
            tc.tile_pool(name="u", bufs=3) as pool_u,
            tc.tile_pool(name="t2s", bufs=2) as pool_t2s,
            tc.tile_pool(name="s2", bufs=3) as pool_s2,
            tc.tile_pool(name="outst", bufs=2) as pool_out,
            tc.tile_pool(name="ps", bufs=8, space="PSUM") as pool_ps,
            tc.tile_pool(name="dram", bufs=1, space="DRAM") as dram,
        ):
            # constants
            w1t = consts.tile([128, 128], fp8, tag="w1")
            nc.sync.dma_start(w1t[:], w1_ext[:])
            w2t = consts.tile([128, 128], bf16, tag="w2")
            nc.sync.dma_start(w2t[:], w2_ext[:])
            b1t = consts.tile([128, F], f32, tag="b1")
            nc.sync.dma_start(b1t[:], b1_ext[:])
            b2t = consts.tile([128, 1], f32, tag="b2")
            nc.sync.dma_start(b2t[:], b2_ext[:])
            dit = consts.tile([128, BPC], f32, tag="di")
            nc.sync.dma_start(dit[:], di_ext[:])
            di8 = consts.tile([128, BPC], f32, tag="di8")
            nc.sync.dma_start(di8[:], d8_ext[:])
            dbt = consts.tile([128, BPC * 128], f32, tag="db")
            nc.sync.dma_start(dbt[:], db_ext[:])

            # DRAM intermediates: per-checkpoint t2 slabs
            t2loc = [dram.tile([2, 128, F], fp8, tag="t2loc",
                               name=f"t2loc{k}") for k in range(NCK)]
            if with_collective:
                t2full = [dram.tile([N_CORES, 2, 128, F], fp8, tag="t2full",
                                    name=f"t2full{k}", addr_space="Shared")
                          for k in range(NCK)]
            else:
                t2full = [dram.tile([N_CORES, 2, 128, F], fp8, tag="t2full",
                                    name=f"t2full{k}") for k in range(NCK)]

            # Checkpoint-major step order: pairs of checkpoint k (j2 % NCK
            # == k) are produced / exchanged / consumed first, so the xwp
            # ring slots, the t2c reads, and the L2 chains all pipeline in
            # the same order.
            ORD = [k + NCK * i for k in range(NCK) for i in range(NB2 // NCK)]

            # ---- W1: xw pair tiles, node-major, fp8 (stays in SBUF) ----
            # All 8 PSUM banks rotate as [128,512] tiles so the
            # mm->drain->reuse cycle (~1.2us) is fully hidden; drains
            # interleave DVE/ACT at a 7:9 ratio to balance the engines.
            # xwp tiles are ALLOCATED in ORD order: ring releases fire in
            # allocation order, and L1 consumes pairs in ORD order, so the
            # t2c generation can reuse slots progressively.
            xwp = [None] * NB2
            for i in range(NB2):
                xwp[ORD[i]] = pool_xwp.tile([128, 2, F], fp8, tag="xwp",
                                            name=f"xwp{ORD[i]}")
            nchunk = 0
            for j2 in range(NB2):
                xb = pool_xb.tile([128, 2, F], fp8, tag="xb",
                                  name=f"xb{j2}")
                nc.sync.dma_start(
                    xb[:], xb_ext[2 * j2:2 * j2 + 2]
                    .rearrange("a p d -> p a d"))
                xw = xwp[j2]
                for k in range(2):
                    for c3 in range(3):
                        ps = pool_ps.tile([128, 512], f32, tag="ps",
                                          name=f"w1p{nchunk}")
                        for p4 in range(4):
                            pl = 4 * c3 + p4
                            nc.tensor.matmul(
                                ps[:, p4 * 128:(p4 + 1) * 128],
                                xb[:, k, pl * 128:(pl + 1) * 128], w1t[:],
                                start=True, stop=True)
                        dst = xw[:, k, c3 * 512:(c3 + 1) * 512]
                        if (nchunk * 29) % 60 < 29:
                            nc.vector.tensor_scalar_mul(dst, ps[:], 1.0)
                        else:
                            nc.scalar.activation(dst, ps[:], AF.Copy)
                        nchunk += 1

            # ---- MT chunk loads (ring bufs=2; ch0/1 prefetch for L1) ----
            mts = []
            for ch in range(NCH):
                mtt = pool_mt.tile([128, NB2, 2, CHW], fp8, tag="mt",
                                   name=f"mt{ch}")
                nc.sync.dma_start(
                    mtt[:].rearrange("p a b d -> p (a b d)"), mt_ext[ch])
                mts.append(mtt)

            # ---- L1: t2 = dinv*relu(dinv/8*(M @ xw) + b1), 5 checkpoints
            # Each checkpoint runs its 2 blocks x 3 chains as 6 live psum
            # chains, pair-inner and skewed, so xwp slots free across the
            # whole pass and chain endings stagger.
            for ch in range(NCK):
                t2st = pool_t2s.tile([128, 2, F], fp8, tag="t2s",
                                     name=f"t2s{ch}")
                ps_list = [pool_ps.tile([128, 512], f32, tag="ps",
                                        name=f"pa{ch}_{idx}")
                           for idx in range(6)]
                for r in range(NB2 + 6):
                    for idx in range(6):
                        i = r - idx
                        if not 0 <= i < NB2:
                            continue
                        j2 = ORD[i]
                        h2, (c0, w) = idx // 3, CHAINS[idx % 3]
                        nc.tensor.matmul(
                            ps_list[idx][:],
                            mts[ch][:, j2, :, h2 * 128:(h2 + 1) * 128],
                            xwp[j2][:, :, c0:c0 + w],
                            start=(i == 0), stop=(i == NB2 - 1),
                            perf_mode=DR)
                    idx = r - (NB2 - 1)
                    if 0 <= idx < 6:
                        h2, (c0, w) = idx // 3, CHAINS[idx % 3]
                        bi = 2 * ch + h2
                        u = pool_u.tile([128, 512], f32, tag="u")
                        nc.vector.scalar_tensor_tensor(
                            u[:], ps_list[idx][:], di8[:, bi:bi + 1],
                            b1t[:, c0:c0 + w],
                            mybir.AluOpType.mult, mybir.AluOpType.add)
                        nc.scalar.activation(t2st[:, h2, c0:c0 + w], u[:],
                                             AF.Relu,
                                             scale=dit[:, bi:bi + 1])
                # distribute this checkpoint's 2 blocks
                nc.gpsimd.dma_start(
                    t2loc[ch][:].rearrange("a p f -> p a f"), t2st[:])
                if with_collective:
                    nc.gpsimd.collective_compute(
                        "AllGather", mybir.AluOpType.bypass,
                        replica_groups=[list(range(N_CORES))],
                        ins=[t2loc[ch][:]], outs=[t2full[ch][:]])
                else:
                    for c in range(N_CORES):
                        nc.gpsimd.dma_start(t2full[ch][c], t2loc[ch][:])

            # ---- MT reload for L2 ch3 first (its slot frees mid-L1) ----
            mtb = {NCH - 1: mts[NCH - 1]}
            t = pool_mt.tile([128, NB2, 2, CHW], fp8, tag="mt",
                             name=f"mtb{NCH - 2}")
            nc.sync.dma_start(
                t[:].rearrange("p a b d -> p (a b d)"), mt_ext[NCH - 2])
            mtb[NCH - 2] = t

            # ---- t2c pair tiles (reuse the xwp ring: t2c[j2] -> slot j2;
            # DMAs issued in ORD order so reads chase the slot releases
            # and the checkpoint arrivals in lock-step) ----
            t2c = [pool_xwp.tile([128, 2, F], fp8, tag="xwp",
                                 name=f"t2c{j2}") for j2 in range(NB2)]
            for i in range(NB2):
                j2 = ORD[i]
                gc = 2 * j2
                core, ck = gc // BPC, (gc % BPC) // 2
                nc.sync.dma_start(
                    t2c[j2][:], t2full[ck][core].rearrange("a p f -> p a f"))

            # ---- remaining MT reloads (descending; ch4 still resident) ----
            for ch in range(NCH - 3, -1, -1):
                t = pool_mt.tile([128, NB2, 2, CHW], fp8, tag="mt",
                                 name=f"mtb{ch}")
                nc.sync.dma_start(
                    t[:].rearrange("p a b d -> p (a b d)"), mt_ext[ch])
                mtb[ch] = t

            # ---- L2 (flipped) + W2 + sigmoid, per dst chunk ----
            # Chains run skewed (chain pl lags pl rounds) so chain endings
            # stagger and the drain/W2/sigmoid tail pipelines instead of
            # serializing after the whole chunk.
            for ch in range(NCH - 1, -1, -1):
                mtt = mtb[ch]
                wt = [pool_ps.tile([128, 512], f32, tag="ps",
                                   name=f"l2w{ch}_{i}") for i in range(6)]
                outst = None
                for r in range(NB2 + PAIRS):
                    for pl in range(PAIRS):
                        i = r - pl
                        if not 0 <= i < NB2:
                            continue
                        j2 = ORD[i]
                        nc.tensor.matmul(
                            wt[pl // 2][:, (pl % 2) * CHW:(pl % 2 + 1) * CHW],
                            t2c[j2][:, :, pl * 128:(pl + 1) * 128],
                            mtt[:, j2, :, :],
                            start=(i == 0), stop=(i == NB2 - 1),
                            perf_mode=DR)
                    pl = r - (NB2 - 1)
                    if 0 <= pl < PAIRS:
                        s2 = pool_s2.tile([128, CHW], bf16, tag="s2")
                        nc.vector.tensor_tensor(
                            s2[:],
                            wt[pl // 2][:, (pl % 2) * CHW:(pl % 2 + 1) * CHW],
                            dbt[:, ch * CHW:(ch + 1) * CHW],
                            mybir.AluOpType.mult)
                        ps2 = pool_ps.tile([128, 512], f32, tag="ps",
                                           name=f"w2p{ch}_{pl}")
                        nc.tensor.matmul(ps2[:, :CHW], w2t[:], s2[:],
                                         start=True, stop=True)
                        if pl % 3 == 0:
                            outst = pool_out.tile([128, 3, CHW], f32,
                                                  tag="outst")
                        nc.scalar.activation(outst[:, pl % 3, :],
                                             ps2[:, :CHW],
                                             AF.Sigmoid, bias=b2t[:])
                        if pl % 3 == 2:
                            nc.gpsimd.dma_start(
                                out_ext[pl - 2:pl + 1, :,
                                        ch * CHW:(ch + 1) * CHW]
                                .rearrange("a p d -> p a d"), outst[:])

    nc.compile()
    return nc


def prepare_inputs(X, edge_index, W1, b1, W2, b2):
    """Host-side graph/layout prep. Returns per-core in_maps."""
    X = np.asarray(X, dtype=np.float32)
    edge_index = np.asarray(edge_index)
    W1 = np.asarray(W1, dtype=np.float32)
    b1 = np.asarray(b1, dtype=np.float32)
    W2 = np.asarray(W2, dtype=np.float32)
    b2 = np.asarray(b2, dtype=np.float32)

    src = edge_index[0].astype(np.int64)
    dst = edge_index[1].astype(np.int64)

    deg = np.bincount(dst, minlength=N).astype(np.float32) + 1.0
    dinv = 1.0 / np.sqrt(deg)
    dinv_pad = np.zeros(NP, np.float32)
    dinv_pad[:N] = dinv

    # M = Adj + I with multiplicity, uint8 counts
    Mfull = np.zeros((NP, NP), np.uint8)
    np.add.at(Mfull, (dst, src), 1)
    Mfull[np.arange(N), np.arange(N)] += 1
    assert Mfull.max() <= 15, "fp8e4 exact-int range exceeded"

    # XB: [NB, 128=(h,cin), PAIRS*128] fp8 with dinv-src folded; s = 2*pl+h
    Xs = X * dinv[None, :, None, None]                  # [B, N, T, C]
    XT = np.zeros((S, C, NP), np.float32)
    XT[:, :, :N] = np.transpose(Xs, (0, 2, 3, 1)).reshape(S, C, N)
    x6 = XT.reshape(PAIRS, 2, C, NB, 128)
    XB = np.ascontiguousarray(np.transpose(x6, (3, 1, 2, 0, 4)))
    XB = XB.reshape(NB, 128, PAIRS * 128).astype(ml_dtypes.float8_e4m3)

    def blockdiag(W, dtype):
        D = np.zeros((128, 128), np.float32)
        D[:64, :64] = W
        D[64:, 64:] = W
        return D.astype(dtype)

    W1d = blockdiag(W1 * W1SCALE, ml_dtypes.float8_e4m3)
    W2d = blockdiag(W2, ml_dtypes.bfloat16)
    B1 = np.tile(b1, (128, F // C)).astype(np.float32)
    B2 = np.concatenate([b2, b2])[:, None].astype(np.float32)

    in_maps = []
    for c in range(N_CORES):
        # MT slab: [NCH][128 src][j2][k][dst-chunk] fp8 ints
        MTc = Mfull[c * BPC * 128:(c + 1) * BPC * 128, :].T  # [NP src, 1280]
        MTc = MTc.reshape(NB2, 2, 128, NCH, CHW)
        MTc = np.ascontiguousarray(np.transpose(MTc, (3, 2, 0, 1, 4)))
        MTc = MTc.reshape(NCH, 128, NB2 * 2 * CHW)
        MTc = MTc.astype(ml_dtypes.float8_e4m3)

        DIc = dinv_pad[c * BPC * 128:(c + 1) * BPC * 128]
        DI = np.ascontiguousarray(DIc.reshape(BPC, 128).T.astype(np.float32))
        DI8 = np.ascontiguousarray(DI / W1SCALE)
        DB = np.ascontiguousarray(
            np.tile(DIc[None, :], (128, 1)).astype(np.float32))
        in_maps.append({"XB8": XB, "MT": MTc, "W1d": W1d, "W2d": W2d,
                        "B1": B1, "B2": B2, "DI": DI, "DI8": DI8,
                        "DB": DB})
    return in_maps


_NC_CACHE = {}


def kernel(X, edge_index, W1, b1, W2, b2):
    if "nc" not in _NC_CACHE:
        _NC_CACHE["nc"] = build_program(with_collective=True)
    nc = _NC_CACHE["nc"]
    in_maps = prepare_inputs(X, edge_index, W1, b1, W2, b2)

    res = None
    for attempt in range(5):
        try:
            res = run_bass_kernel_spmd(nc, in_maps, list(range(N_CORES)))
            break
        except Exception:
            if attempt == 4:
                raise
            time.sleep(60.0 * (attempt + 1))
    assert res is not None

    # reassemble: per core [12, 128, 1280] -> [24, 64, 1280]
    full = np.zeros((S, C, N), np.float32)
    for c in range(N_CORES):
        o = res.results[c]["OUT"].reshape(S, C, BPC * 128)
        lo = c * BPC * 128
        hi = min(N, (c + 1) * BPC * 128)
        if lo < N:
            full[:, :, lo:hi] = o[:, :, :hi - lo]
    out = full.reshape(B, T, C, N).transpose(0, 3, 1, 2)
    return np.ascontiguousarray(out)
